# revision 1
# baseline (speedup 1.0000x reference)
"""Bilateral-solver local loss on 8 TRN2 NeuronCores (Bass/Tile, SPMD).

loss = H*W*LAM * mean(w_ij * d^2) + mean((output-target)^2),
d[k] = output - shift_k(output) over the 440 non-center 21x21 offsets
(replicate padding).

Sharding: the 21 di-rows of the offset grid are distributed over cores
(3 rows per core; core 7 gets empty zero-weight rows). All cores run an
IDENTICAL program; which global (di, dj) a core's slot means is decided
entirely by host-side data prep: the host uploads per-(stripe, di) crops
("slabs") of the padded image, plus a w buffer reordered to match the
kernel's tile layout, with unused slots zero-weighted.

Per core: S_c = sum(w * (x - shift(x))^2) over its slots, computed as
DVE subtract -> ACT square -> DVE multiply -> PE ones-matmul reduction
accumulated into PSUM. The data term is computed on every core (core 0's
is used). Host combines: loss = LAM/440 * sum_c S_c + D / (H*W).
"""

import sys

for _p in ("/opt/trn_rl_repo", "/root/.axon_site/_ro/trn_rl_repo"):
    if _p not in sys.path:
        sys.path.append(_p)

import numpy as np

H = W = 320
K = 21
P = 10
LAM = 128.0
NOFF = 440
N_CORES = 8

NSTRIPE = 3
RS = 108           # output rows per stripe (stripe 2: 104 real + 4 zero pad)
NDI = 3            # local di rows per core
HALF_DJS = [(0, 11), (11, 10)]   # (dj0, g) free-dim dj groups
SLAB_COLS = 344
NCHUNK = 8         # matmul chunks per (slot, half)
DI0 = [0, 3, 6, 9, 12, 15, 18, 21]  # global di base per core

_CACHE = {}


def _build_program():
    import bass_rust
    import concourse.bacc as bacc
    import concourse.mybir as mybir
    import concourse.tile as tile

    nc = bacc.Bacc("TRN2", target_bir_lowering=False, debug=False,
                   num_devices=N_CORES)
    f32 = mybir.dt.float32

    slab_d = nc.dram_tensor("slab", [NSTRIPE * NDI, RS, SLAB_COLS], f32,
                            kind="ExternalInput")
    xs_d = nc.dram_tensor("xs", [NSTRIPE, RS, W], f32, kind="ExternalInput")
    wa_d = nc.dram_tensor("wa", [NSTRIPE, NDI, RS, 11 * W], f32,
                          kind="ExternalInput")
    wb_d = nc.dram_tensor("wb", [NSTRIPE, NDI, RS, 10 * W], f32,
                          kind="ExternalInput")
    o_d = nc.dram_tensor("o", [H, W], f32, kind="ExternalInput")
    t_d = nc.dram_tensor("t", [H, W], f32, kind="ExternalInput")
    out_d = nc.dram_tensor("out", [1, 2], f32, kind="ExternalOutput")

    with tile.TileContext(nc) as tc:
        with (
            tc.tile_pool(name="const", bufs=1) as cpool,
            tc.tile_pool(name="slab", bufs=2) as slabpool,
            tc.tile_pool(name="xs", bufs=2) as xspool,
            tc.tile_pool(name="w", bufs=3) as wpool,
            tc.tile_pool(name="d", bufs=2) as dpool,
            tc.tile_pool(name="d2", bufs=2) as d2pool,
            tc.tile_pool(name="t", bufs=2) as tpool,
            tc.tile_pool(name="small", bufs=1) as smallpool,
            tc.tile_pool(name="psum", bufs=1, space="PSUM") as psumpool,
        ):
            ones = cpool.tile([128, 1], f32)
            nc.vector.memset(ones[:], 1.0)

            acc = psumpool.tile([1, 512], f32)
            mm_i = 0
            total_mms = NSTRIPE * NDI * len(HALF_DJS) * NCHUNK

            for s in range(NSTRIPE):
                xs_t = xspool.tile([RS, W], f32)
                nc.sync.dma_start(xs_t[:], xs_d[s])
                for dl in range(NDI):
                    slab_t = slabpool.tile([RS, SLAB_COLS], f32, tag="slab")
                    nc.sync.dma_start(slab_t[:], slab_d[s * NDI + dl])
                    for half, (dj0, g) in enumerate(HALF_DJS):
                        fd = g * W
                        w_t = wpool.tile([RS, fd], f32, tag="w")
                        nc.sync.dma_start(
                            w_t[:], (wa_d if half == 0 else wb_d)[s, dl])

                        d_t = dpool.tile([RS, fd], f32, tag="d")
                        # x broadcast over the g dj values: [RS, g(step0), W]
                        in0 = xs_t[0:RS, 0:W].copy()
                        pstep = in0.ap[0][0]
                        in0.ap = bass_rust.VecI64Pair(
                            [(pstep, RS), (0, g), (1, W)])
                        # shifted windows: slab cols dj0+jj .. +W
                        in1 = slab_t[0:RS, dj0:dj0 + 1].copy()
                        pstep1 = in1.ap[0][0]
                        in1.ap = bass_rust.VecI64Pair(
                            [(pstep1, RS), (1, g), (1, W)])
                        out3 = d_t[:].rearrange("p (g w) -> p g w", g=g)
                        nc.vector.tensor_sub(out3, in0, in1)

                        d2_t = d2pool.tile([RS, fd], f32, tag="d2")
                        nc.scalar.activation(
                            d2_t[:], d_t[:],
                            mybir.ActivationFunctionType.Square)
                        t_t = tpool.tile([RS, fd], f32, tag="t")
                        nc.vector.tensor_mul(t_t[:], w_t[:], d2_t[:])

                        ck = fd // NCHUNK
                        for j in range(NCHUNK):
                            nc.tensor.matmul(
                                acc[0:1, 0:ck],
                                ones[0:RS, :],
                                t_t[:, j * ck:(j + 1) * ck],
                                start=(mm_i == 0),
                                stop=(mm_i == total_mms - 1),
                            )
                            mm_i += 1
            assert mm_i == total_mms

            # data term sum((o-t)^2), flat [128, 800]
            of = o_d.ap().flatten().rearrange("(p f) -> p f", p=128)
            tf = t_d.ap().flatten().rearrange("(p f) -> p f", p=128)
            o_t = smallpool.tile([128, 800], f32, tag="o")
            t_t2 = smallpool.tile([128, 800], f32, tag="t2")
            nc.sync.dma_start(o_t[:], of)
            nc.sync.dma_start(t_t2[:], tf)
            dt_t = smallpool.tile([128, 800], f32, tag="dt")
            nc.vector.tensor_sub(dt_t[:], o_t[:], t_t2[:])
            dt2_t = smallpool.tile([128, 800], f32, tag="dt2")
            nc.scalar.activation(dt2_t[:], dt_t[:],
                                 mybir.ActivationFunctionType.Square)
            dtv = smallpool.tile([128, 1], f32, tag="dtv")
            nc.vector.reduce_sum(dtv[:], dt2_t[:], axis=mybir.AxisListType.X)
            acc2 = psumpool.tile([1, 1], f32)
            nc.tensor.matmul(acc2[:], ones[:, :], dtv[:], start=True,
                             stop=True)

            res = smallpool.tile([1, 2], f32, tag="res")
            nc.vector.reduce_sum(res[0:1, 0:1], acc[0:1, 0:440],
                                 axis=mybir.AxisListType.X)
            nc.vector.tensor_copy(res[0:1, 1:2], acc2[0:1, 0:1])
            nc.sync.dma_start(out_d[:], res[:])

    nc.compile()
    return nc


def get_program():
    if "nc" not in _CACHE:
        _CACHE["nc"] = _build_program()
    return _CACHE["nc"]


def host_prep(output, target, w_ij):
    """Build the 8 per-core input maps."""
    x = np.ascontiguousarray(output, dtype=np.float32)
    tgt = np.ascontiguousarray(target, dtype=np.float32)
    w_ij = np.ascontiguousarray(w_ij, dtype=np.float32)

    padded = np.pad(x, P, mode="edge")  # [340, 340]
    padded_ext = np.zeros((364, SLAB_COLS), dtype=np.float32)
    padded_ext[:340, :340] = padded

    xs = np.zeros((NSTRIPE, RS, W), dtype=np.float32)
    for s in range(NSTRIPE):
        r0 = RS * s
        n = min(RS, H - r0)
        xs[s, :n] = x[r0:r0 + n, :]

    # w with the center offset re-inserted as zeros -> [21, 21, H, W] view
    w_full = np.zeros((K * K, H, W), dtype=np.float32)
    w_full[:220] = w_ij[:220]
    w_full[221:] = w_ij[220:]
    w_full = w_full.reshape(K, K, H, W)

    in_maps = []
    for c in range(N_CORES):
        di0 = DI0[c]
        slabs = np.zeros((NSTRIPE * NDI, RS, SLAB_COLS), dtype=np.float32)
        wa = np.zeros((NSTRIPE, NDI, RS, 11 * W), dtype=np.float32)
        wb = np.zeros((NSTRIPE, NDI, RS, 10 * W), dtype=np.float32)
        for s in range(NSTRIPE):
            r0 = RS * s
            n = min(RS, H - r0)
            for dl in range(NDI):
                di = di0 + dl
                slabs[s * NDI + dl] = padded_ext[r0 + di:r0 + di + RS, :]
                if di < K:
                    blk = w_full[di, :, r0:r0 + n, :]        # [21, n, W]
                    blk = blk.transpose(1, 0, 2)             # [n, 21, W]
                    wa[s, dl, :n] = blk[:, :11, :].reshape(n, 11 * W)
                    wb[s, dl, :n] = blk[:, 11:, :].reshape(n, 10 * W)
        in_maps.append({
            "slab": slabs, "xs": xs, "wa": wa, "wb": wb,
            "o": x, "t": tgt,
        })
    return in_maps


def combine(results):
    S = 0.0
    for c in range(N_CORES):
        S += float(results[c]["out"][0, 0])
    D = float(results[0]["out"][0, 1])
    loss = (LAM / NOFF) * S + D / (H * W)
    return np.array(loss, dtype=np.float32)


def kernel(output, target, w_ij):
    from concourse.bass_utils import run_bass_kernel_spmd

    nc = get_program()
    in_maps = host_prep(output, target, w_ij)
    res = run_bass_kernel_spmd(nc, in_maps, list(range(N_CORES)))
    return combine(res.results)


if __name__ == "__main__":
    rng = np.random.default_rng(0)
    output = rng.random((H, W), dtype=np.float32)
    target = rng.random((H, W), dtype=np.float32)
    w_ij = rng.random((NOFF, H, W), dtype=np.float32)
    got = kernel(output=output, target=target, w_ij=w_ij)

    padded = np.pad(np.float64(output), P, mode="edge")
    S = 0.0
    for di in range(K):
        for dj in range(K):
            if di == P and dj == P:
                continue
            k = di * K + dj - (1 if di * K + dj > 220 else 0)
            d = output - padded[di:di + H, dj:dj + W]
            S += float((np.float64(w_ij[k]) * d * d).sum())
    D = float((np.float64(output - target) ** 2).sum())
    exp = (LAM / NOFF) * S + D / (H * W)
    print("got:", got, "expected:", exp, "rel err:",
          abs(float(got) - exp) / abs(exp))
